# revision 24
# baseline (speedup 1.0000x reference)
"""Trainium2 Bass kernel: 3x3 conv (NCHW 32x256x56x56, 256->256ch, pad 1) with
host-expanded synthesized weight, data-parallel over 8 NeuronCores.

1D Winograd F(2,3) along W cuts PE matmul work to 2/3 of direct implicit
GEMM: per 2 output columns, 4 winograd positions x (3 dy x 2 ch-tiles)
= 24 col-cycles vs 36 direct.  PE floor ~125us vs ~188us direct fp16
(fp8 DoubleRow would be 2x faster still but e4m3 on even one conv
operand measures 2.7e-2 max-rel error vs the 2e-2 gate - dead end).

Pipeline per image (4 per core): input arrives as column-parity planes
(even/odd/even-shifted, host-prepared) so DVE's 4 transform ops per
(kt, row-band) T0=d0-d2, T1=d1+d2, T2=d2-d1, T3=d1-d3 are unit-stride
fp16 and mostly run in the 2x packed mode; PE accumulates
M_j[co, r, t] = sum_{kt,dy} Gw[:,dy,j]^T T_j[:, r+dy, :] into 4 PSUM
banks per (14-row chunk, mt), 2 chunk-mt in flight (8 banks); ACT
stages the banks to fp16 SBUF (bias folded into M1, which carries a +
sign in both output phases); DVE finishes the inverse transform
y_even = M0+(M1+b)+M2, y_odd = (M1+b)-M2-M3 with 4 all-fp16 2x ops
into a phase-split fp16 out tile; host interleaves phases + casts fp32.

Measured ~163-170us per core (baseline direct implicit GEMM: ~228us);
PE matmul stream ~134us (84% occupancy), DVE ~90us, ACT ~75us, plus
~7.5us fixed NEFF preamble and ~12us inverse/DMA-drain/teardown tail.
"""

import numpy as np

# Problem constants (hardcoded per contract; kernel.py must be self-contained)
OOC, OIC, K1, K2 = 64, 64, 3, 3
R0, R1 = 4, 4
N_CORES = 8
BATCH = 32
N_PER_CORE = BATCH // N_CORES  # 4
C = 256
H = W = 56
HP = WP = H + 2  # zero-padded spatial (padding applied on host)
KT = C // 128    # 2 input-channel tiles
MT = C // 128    # 2 output-channel tiles
NT = 28          # winograd tiles along W (4-wide windows, stride 2)
NJ = 4           # winograd positions
RB = 14          # output rows per chunk -> psum bank [128, 14, 28] = 392 fp32
NCH = H // RB    # 4 chunks
# transform/DMA row bands (transform is row-local so any partition works);
# image 0 uses 4 chunk-aligned bands for fast start, later images 3 bands
# spread across chunk slots so input DMA (~19us/image) hides behind compute
BANDS4 = [(0, 16), (16, 14), (30, 14), (44, 14)]
TB_A0, TB_A1, TB_B = (0, 16), (16, 14), (30, 28)
DMA_BANDS = [TB_A0, TB_A1, TB_B]

_NC_CACHE = {}
LAST_RESULT = {}  # test.py introspection: last BassKernelResults

# F(2,3): G rows -> Gw_j weights; B^T rows -> T_j transforms
_G = np.array([[1, 0, 0], [0.5, 0.5, 0.5], [0.5, -0.5, 0.5], [0, 0, 1]],
              dtype=np.float32)


def _expand_weight(weight, alphas, betas):
    """W[p0*64+i, p1*64+j, ky, kx] = w[i,j,ky,kx] * a[p0,p1] / (1+exp(w*b[p0,p1]))."""
    w = weight.astype(np.float32)[None, None]            # (1,1,64,64,3,3)
    a = alphas.astype(np.float32).reshape(R0, R1)[:, :, None, None, None, None]
    b = betas.astype(np.float32).reshape(R0, R1)[:, :, None, None, None, None]
    act = w * a / (1.0 + np.exp(w * b))                  # (4,4,64,64,3,3)
    return act.transpose(0, 2, 1, 3, 4, 5).reshape(R0 * OOC, R1 * OIC, K1, K2)


def _host_prep(x, weight, alphas, betas, bias):
    x = np.asarray(x, dtype=np.float32).astype(np.float16)
    xpad = np.pad(x, ((0, 0), (0, 0), (1, 1), (1, 1)))
    xpad = xpad.reshape(BATCH, KT, 128, HP, WP)
    # column-parity planes, 30 elems each (4B-aligned unit-stride taps so
    # every DVE transform op hits the 2x packed mode):
    #   plane0 = even cols e0.., plane1 = odd cols o0..,
    #   plane2 = even shifted left (e1..), plane3 = odd shifted (o1..)
    xp3 = np.zeros((BATCH, KT, 128, HP, 3, 30), np.float16)
    xp3[..., 0, :29] = xpad[..., 0::2]
    xp3[..., 1, :29] = xpad[..., 1::2]
    xp3[..., 2, :28] = xpad[..., 2::2]
    xpad = xp3
    Wfull = _expand_weight(np.asarray(weight), np.asarray(alphas),
                           np.asarray(betas))            # (256,256,3,3) co,ci
    # winograd weights: Gw[ci, co, dy, j] = sum_k G[j,k] * W[co, ci, dy, k]
    Wt = Wfull.transpose(1, 0, 2, 3)                     # (ci, co, dy, kx)
    Gw = np.einsum("jk,iodk->iodj", _G, Wt)              # (ci, co, dy, 4)
    # lhsT layout: [ci_local(128), kt, mt, dy, j, co_local(128)]
    w_arr = np.ascontiguousarray(
        Gw.reshape(KT, 128, MT, 128, K1, NJ).transpose(1, 0, 2, 4, 5, 3)
    ).astype(np.float16)
    b_arr = np.ascontiguousarray(
        np.asarray(bias, dtype=np.float32).reshape(MT, 128).T)
    return xpad, w_arr, b_arr


def _build_nc():
    import concourse.mybir as mybir
    import concourse.tile as tile
    from concourse import bacc

    fp32 = mybir.dt.float32
    fp16 = mybir.dt.float16

    nc = bacc.Bacc("TRN2", target_bir_lowering=False, debug=False,
                   num_devices=N_CORES)

    x_d = nc.dram_tensor("x", [N_PER_CORE, KT, 128, HP, 3, 30], fp16,
                         kind="ExternalInput")
    w_d = nc.dram_tensor("w", [128, KT, MT, K1, NJ, 128], fp16,
                         kind="ExternalInput")
    b_d = nc.dram_tensor("b", [128, MT], fp32, kind="ExternalInput")
    o_d = nc.dram_tensor("out", [N_PER_CORE, MT, NCH, 128, RB, 2, NT], fp16,
                         kind="ExternalOutput")

    # Two HWDGE rings: sync carries kt=0 traffic, scalar carries kt=1.
    def ring(i):
        return nc.sync if i == 0 else nc.scalar

    with tile.TileContext(nc) as tc:
        with (
            tc.tile_pool(name="const", bufs=1) as const_pool,
            tc.tile_pool(name="xpad", bufs=1) as xp_pool,
            tc.tile_pool(name="tw", bufs=1) as tw_pool,
            tc.tile_pool(name="tmp", bufs=4) as tmp_pool,
            tc.tile_pool(name="ot", bufs=8) as out_pool,
            tc.tile_pool(name="ps", bufs=8, space="PSUM") as psum_pool,
        ):
            w_sb = const_pool.tile([128, KT, MT, K1, NJ, 128], fp16,
                                   name="w_sb", tag="w_sb")
            b_sb = const_pool.tile([128, MT], fp32, name="b_sb", tag="b_sb")

            # PE warmup: junk matmuls on scratch SBUF while the head DMAs
            # land flip the HAM clock gate to 8/8 before the real stream.
            warm_in = const_pool.tile([128, 128], fp16, name="warm_in",
                                      tag="warm_in")
            warm_ps = psum_pool.tile([128, RB, NT], fp32, name="warm_ps",
                                     tag="ps")
            nc.vector.memset(warm_in[:], 0.0)
            for _ in range(160):
                nc.tensor.matmul(warm_ps[:, 0, :], warm_in[:],
                                 warm_in[:, 0:NT])

            # Double-buffered padded input images and winograd-domain input.
            # T layout: [128, kt, j, 58 rows, 28 tiles] fp16.
            xp = [xp_pool.tile([128, KT, HP, 3, 30], fp16, name=f"xp{par}",
                               tag=f"xp{par}") for par in range(2)]
            tws = [tw_pool.tile([128, KT, NJ, HP, NT], fp16, name=f"tw{par}",
                                tag=f"tw{par}") for par in range(2)]

            xap = x_d.ap()
            oap = o_d.ap()

            def band_dma(n, par, r0, nr):
                for kt in range(KT):
                    ring(kt).dma_start(
                        xp[par][:, kt, r0:r0 + nr, :, :],
                        xap[n, kt, :, r0:r0 + nr, :, :])

            def transform_band(par, r0, nr):
                tw = tws[par]
                for kt in range(KT):
                    # parity planes: d0 = even cols, d1 = odd, d2 = even
                    # shifted left; those taps are unit-stride 4B-aligned
                    # so their DVE ops pack 2 fp16/cycle.  d3 reuses the
                    # odd plane at +1 (misaligned, slower op) - cheaper
                    # than a 4th plane's DMA volume or GpSimd's SBUF-port
                    # contention with DVE.
                    xk = xp[par]
                    d = [xk[:, kt, r0:r0 + nr, 0, 0:NT],
                         xk[:, kt, r0:r0 + nr, 1, 0:NT],
                         xk[:, kt, r0:r0 + nr, 2, 0:NT],
                         xk[:, kt, r0:r0 + nr, 1, 1:NT + 1]]
                    o = [tw[:, kt, j, r0:r0 + nr, :] for j in range(NJ)]
                    nc.vector.tensor_sub(o[0], d[0], d[2])   # T0 = d0 - d2
                    nc.vector.tensor_add(o[1], d[1], d[2])   # T1 = d1 + d2
                    nc.vector.tensor_sub(o[2], d[2], d[1])   # T2 = d2 - d1
                    nc.vector.tensor_sub(o[3], d[1], d[3])   # T3 = d1 - d3

            # Head: image-0 bands interleaved with per-(mt,j) weight DMAs
            # in exactly the order the PE consumes them, racing PE warmup.
            wap = w_d.ap()

            def w_dma(mt, j):
                for kt in range(KT):
                    ring(kt).dma_start(w_sb[:, kt, mt, :, j],
                                       wap[:, kt, mt, :, j])

            band_dma(0, 0, *BANDS4[0])
            for j in range(NJ):
                w_dma(0, j)
            w_dma(1, 0)
            w_dma(1, 1)
            band_dma(0, 0, *BANDS4[1])
            w_dma(1, 2)
            w_dma(1, 3)
            band_dma(0, 0, *BANDS4[2])
            band_dma(0, 0, *BANDS4[3])
            nc.scalar.dma_start(b_sb[:], b_d.ap())
            transform_band(0, *BANDS4[0])

            for n in range(N_PER_CORE):
                par = n % 2
                npar = (n + 1) % 2
                nxt = n + 1 < N_PER_CORE
                chunks = [(c, c * RB, RB) for c in range(NCH)]
                if n == N_PER_CORE - 1:
                    # split the final chunk so the closing inverse + out-DMA
                    # drain after the last matmul is half as long
                    chunks = chunks[:-1] + [(NCH - 1, 42, 7), (NCH - 1, 49, 7)]
                for c, y0, rb in chunks:
                    # just-in-time transforms / prefetch DMAs, interleaved
                    # into the DVE stream between inverse-transform ops:
                    # rows 0..29 of image n are transformed during image
                    # n-1 (c2/c3 slots), rows 30..57 at (n, c0).  Keeping
                    # transforms early also releases the xp WAR quickly so
                    # the next image's DMA enqueues never head-of-line
                    # block the ACT queue.
                    if c == 0:
                        if n == 0:
                            transform_band(0, *BANDS4[1])
                        else:
                            transform_band(par, *TB_B)
                        if nxt:
                            for bnd in DMA_BANDS:
                                band_dma(n + 1, npar, *bnd)
                    elif c == 1:
                        if n == 0:
                            transform_band(0, *BANDS4[2])
                    elif c == 2:
                        if n == 0:
                            transform_band(0, *BANDS4[3])
                        if nxt:
                            transform_band(npar, *TB_A0)
                    elif c == 3 and nxt:
                        transform_band(npar, *TB_A1)
                    for mt in range(MT):
                        ps = [psum_pool.tile([128, RB, NT], fp32,
                                             name="ps", tag="ps")
                              for _ in range(NJ)]
                        for j in range(NJ):
                            first = True
                            for kt in range(KT):
                                for dy in range(K1):
                                    last = (kt == KT - 1 and dy == K1 - 1)
                                    nc.tensor.matmul(
                                        ps[j][:, 0:rb, :],
                                        w_sb[:, kt, mt, dy, j, :],
                                        tws[par][:, kt, j,
                                                 y0 + dy:y0 + dy + rb, :],
                                        start=first, stop=last,
                                    )
                                    first = False
                        # inverse transform + bias.  ACT stages the PSUM
                        # banks to fp16 SBUF (bias folded into M1, which
                        # appears with + sign in both output phases); DVE
                        # then runs 4 all-fp16 unit-stride ops at 2x:
                        #   y_even = M0 + (M1+b) + M2
                        #   y_odd  = (M1+b) - M2 - M3
                        # ot is phase-split [rows, 2, 28]; host interleaves.
                        ot = out_pool.tile([128, RB, 2, NT], fp16,
                                           name="ot", tag="ot")
                        cc = [tmp_pool.tile([128, RB, NT], fp16,
                                            name=f"c{k}", tag=f"c{k}")
                              for k in range(NJ)]
                        nc.scalar.copy(cc[0][:, 0:rb], ps[0][:, 0:rb])
                        nc.scalar.add(cc[1][:, 0:rb], ps[1][:, 0:rb],
                                      b_sb[:, mt:mt + 1])
                        nc.scalar.copy(cc[2][:, 0:rb], ps[2][:, 0:rb])
                        nc.scalar.copy(cc[3][:, 0:rb], ps[3][:, 0:rb])
                        e1 = tmp_pool.tile([128, RB, NT], fp16,
                                           name="e1", tag="e1")
                        u1 = tmp_pool.tile([128, RB, NT], fp16,
                                           name="u1", tag="u1")
                        nc.vector.tensor_add(e1[:, 0:rb], cc[0][:, 0:rb],
                                             cc[1][:, 0:rb])
                        nc.vector.tensor_add(ot[:, 0:rb, 0, :], e1[:, 0:rb],
                                             cc[2][:, 0:rb])
                        nc.vector.tensor_sub(u1[:, 0:rb], cc[1][:, 0:rb],
                                             cc[2][:, 0:rb])
                        nc.vector.tensor_sub(ot[:, 0:rb, 1, :], u1[:, 0:rb],
                                             cc[3][:, 0:rb])
                        # out DMA: mt0 on the sync ring, mt1 on the gpsimd
                        # ring (keeps the ACT queue free of DMA waits; only
                        # SP/Activation/gpsimd can initiate DMAs)
                        r0 = y0 - c * RB
                        (nc.sync if mt == 0 else nc.gpsimd).dma_start(
                            oap[n, mt, c, :, r0:r0 + rb], ot[:, 0:rb])
    nc.compile()
    return nc


def get_nc():
    if "nc" not in _NC_CACHE:
        _NC_CACHE["nc"] = _build_nc()
    return _NC_CACHE["nc"]


def kernel(x, weight, alphas, betas, bias):
    from concourse.bass_utils import run_bass_kernel_spmd

    xpad, w_arr, b_arr = _host_prep(x, weight, alphas, betas, bias)
    nc = get_nc()
    in_maps = [
        {"x": xpad[i * N_PER_CORE:(i + 1) * N_PER_CORE], "w": w_arr,
         "b": b_arr}
        for i in range(N_CORES)
    ]
    res = run_bass_kernel_spmd(nc, in_maps, core_ids=list(range(N_CORES)))
    LAST_RESULT["res"] = res
    out = np.concatenate([r["out"] for r in res.results], axis=0)
    # [32, MT, NCH, 128, RB, 2, NT] -> [32, 256, 56, 56]
    # (x = 2*t + phase: transpose phase behind the tile index)
    return np.ascontiguousarray(
        out.transpose(0, 1, 3, 2, 4, 6, 5).reshape(BATCH, C, H, W)
    ).astype(np.float32)


# revision 25
# speedup vs baseline: 1.0108x; 1.0108x over previous
"""Trainium2 Bass kernel: 3x3 conv (NCHW 32x256x56x56, 256->256ch, pad 1) with
host-expanded synthesized weight, data-parallel over 8 NeuronCores.

1D Winograd F(2,3) along W cuts PE matmul work to 2/3 of direct implicit
GEMM: per 2 output columns, 4 winograd positions x (3 dy x 2 ch-tiles)
= 24 col-cycles vs 36 direct.  PE floor ~125us vs ~188us direct fp16
(fp8 DoubleRow would be 2x faster still but e4m3 on even one conv
operand measures 2.7e-2 max-rel error vs the 2e-2 gate - dead end).

Pipeline per image (4 per core): input arrives as column-parity planes
(even/odd/even-shifted, host-prepared) so DVE's 4 transform ops per
(kt, row-band) T0=d0-d2, T1=d1+d2, T2=d2-d1, T3=d1-d3 are unit-stride
fp16 and mostly run in the 2x packed mode; PE accumulates
M_j[co, r, t] = sum_{kt,dy} Gw[:,dy,j]^T T_j[:, r+dy, :] into 4 PSUM
banks per (14-row chunk, mt), 2 chunk-mt in flight (8 banks); ACT
stages the banks to fp16 SBUF (bias folded into M1, which carries a +
sign in both output phases); DVE finishes the inverse transform
y_even = M0+(M1+b)+M2, y_odd = (M1+b)-M2-M3 with 4 all-fp16 2x ops
into a phase-split fp16 out tile; host interleaves phases + casts fp32.

Measured ~163-170us per core (baseline direct implicit GEMM: ~228us);
PE matmul stream ~134us (84% occupancy), DVE ~90us, ACT ~75us, plus
~7.5us fixed NEFF preamble and ~12us inverse/DMA-drain/teardown tail.
"""

import numpy as np

# Problem constants (hardcoded per contract; kernel.py must be self-contained)
OOC, OIC, K1, K2 = 64, 64, 3, 3
R0, R1 = 4, 4
N_CORES = 8
BATCH = 32
N_PER_CORE = BATCH // N_CORES  # 4
C = 256
H = W = 56
HP = WP = H + 2  # zero-padded spatial (padding applied on host)
KT = C // 128    # 2 input-channel tiles
MT = C // 128    # 2 output-channel tiles
NT = 28          # winograd tiles along W (4-wide windows, stride 2)
NJ = 4           # winograd positions
RB = 14          # output rows per chunk -> psum bank [128, 14, 28] = 392 fp32
NCH = H // RB    # 4 chunks
# transform/DMA row bands (transform is row-local so any partition works);
# image 0 uses 4 chunk-aligned bands for fast start, later images 3 bands
# spread across chunk slots so input DMA (~19us/image) hides behind compute
BANDS4 = [(0, 16), (16, 14), (30, 14), (44, 14)]
TB_A0, TB_A1, TB_B = (0, 16), (16, 14), (30, 28)
DMA_BANDS = [TB_A0, TB_A1, TB_B]

_NC_CACHE = {}
LAST_RESULT = {}  # test.py introspection: last BassKernelResults

# F(2,3): G rows -> Gw_j weights; B^T rows -> T_j transforms
_G = np.array([[1, 0, 0], [0.5, 0.5, 0.5], [0.5, -0.5, 0.5], [0, 0, 1]],
              dtype=np.float32)


def _expand_weight(weight, alphas, betas):
    """W[p0*64+i, p1*64+j, ky, kx] = w[i,j,ky,kx] * a[p0,p1] / (1+exp(w*b[p0,p1]))."""
    w = weight.astype(np.float32)[None, None]            # (1,1,64,64,3,3)
    a = alphas.astype(np.float32).reshape(R0, R1)[:, :, None, None, None, None]
    b = betas.astype(np.float32).reshape(R0, R1)[:, :, None, None, None, None]
    act = w * a / (1.0 + np.exp(w * b))                  # (4,4,64,64,3,3)
    return act.transpose(0, 2, 1, 3, 4, 5).reshape(R0 * OOC, R1 * OIC, K1, K2)


def _host_prep(x, weight, alphas, betas, bias):
    x = np.asarray(x, dtype=np.float32).astype(np.float16)
    xpad = np.pad(x, ((0, 0), (0, 0), (1, 1), (1, 1)))
    xpad = xpad.reshape(BATCH, KT, 128, HP, WP)
    # column-parity planes, 30 elems each (4B-aligned unit-stride taps so
    # every DVE transform op hits the 2x packed mode):
    #   plane0 = even cols e0.., plane1 = odd cols o0..,
    #   plane2 = even shifted left (e1..), plane3 = odd shifted (o1..)
    xp3 = np.zeros((BATCH, KT, 128, HP, 3, 30), np.float16)
    xp3[..., 0, :29] = xpad[..., 0::2]
    xp3[..., 1, :29] = xpad[..., 1::2]
    xp3[..., 2, :28] = xpad[..., 2::2]
    xpad = xp3
    Wfull = _expand_weight(np.asarray(weight), np.asarray(alphas),
                           np.asarray(betas))            # (256,256,3,3) co,ci
    # winograd weights: Gw[ci, co, dy, j] = sum_k G[j,k] * W[co, ci, dy, k]
    Wt = Wfull.transpose(1, 0, 2, 3)                     # (ci, co, dy, kx)
    Gw = np.einsum("jk,iodk->iodj", _G, Wt)              # (ci, co, dy, 4)
    # lhsT layout: [ci_local(128), kt, mt, dy, j, co_local(128)]
    w_arr = np.ascontiguousarray(
        Gw.reshape(KT, 128, MT, 128, K1, NJ).transpose(1, 0, 2, 4, 5, 3)
    ).astype(np.float16)
    b_arr = np.ascontiguousarray(
        np.asarray(bias, dtype=np.float32).reshape(MT, 128).T)
    return xpad, w_arr, b_arr


def _build_nc():
    import concourse.mybir as mybir
    import concourse.tile as tile
    from concourse import bacc

    fp32 = mybir.dt.float32
    fp16 = mybir.dt.float16

    nc = bacc.Bacc("TRN2", target_bir_lowering=False, debug=False,
                   num_devices=N_CORES)

    x_d = nc.dram_tensor("x", [N_PER_CORE, KT, 128, HP, 3, 30], fp16,
                         kind="ExternalInput")
    w_d = nc.dram_tensor("w", [128, KT, MT, K1, NJ, 128], fp16,
                         kind="ExternalInput")
    b_d = nc.dram_tensor("b", [128, MT], fp32, kind="ExternalInput")
    o_d = nc.dram_tensor("out", [N_PER_CORE, MT, NCH, 128, RB, 2, NT], fp16,
                         kind="ExternalOutput")

    # Two HWDGE rings: sync carries kt=0 traffic, scalar carries kt=1.
    def ring(i):
        return nc.sync if i == 0 else nc.scalar

    with tile.TileContext(nc) as tc:
        with (
            tc.tile_pool(name="const", bufs=1) as const_pool,
            tc.tile_pool(name="xpad", bufs=1) as xp_pool,
            tc.tile_pool(name="tw", bufs=1) as tw_pool,
            tc.tile_pool(name="tmp", bufs=4) as tmp_pool,
            tc.tile_pool(name="ot", bufs=8) as out_pool,
            tc.tile_pool(name="ps", bufs=8, space="PSUM") as psum_pool,
        ):
            w_sb = const_pool.tile([128, KT, MT, K1, NJ, 128], fp16,
                                   name="w_sb", tag="w_sb")
            b_sb = const_pool.tile([128, MT], fp32, name="b_sb", tag="b_sb")

            # PE warmup: junk matmuls on scratch SBUF while the head DMAs
            # land flip the HAM clock gate to 8/8 before the real stream.
            warm_in = const_pool.tile([128, 128], fp16, name="warm_in",
                                      tag="warm_in")
            warm_ps = psum_pool.tile([128, RB, NT], fp32, name="warm_ps",
                                     tag="ps")
            nc.vector.memset(warm_in[:], 0.0)
            for _ in range(160):
                nc.tensor.matmul(warm_ps[:, 0, :], warm_in[:],
                                 warm_in[:, 0:NT])

            # Double-buffered padded input images and winograd-domain input.
            # T layout: [128, kt, j, 58 rows, 28 tiles] fp16.
            xp = [xp_pool.tile([128, KT, HP, 3, 30], fp16, name=f"xp{par}",
                               tag=f"xp{par}") for par in range(2)]
            tws = [tw_pool.tile([128, KT, NJ, HP, NT], fp16, name=f"tw{par}",
                                tag=f"tw{par}") for par in range(2)]

            xap = x_d.ap()
            oap = o_d.ap()

            def band_dma(n, par, r0, nr):
                for kt in range(KT):
                    ring(kt).dma_start(
                        xp[par][:, kt, r0:r0 + nr, :, :],
                        xap[n, kt, :, r0:r0 + nr, :, :])

            def transform_band(par, r0, nr):
                tw = tws[par]
                for kt in range(KT):
                    # parity planes: d0 = even cols, d1 = odd, d2 = even
                    # shifted left; those taps are unit-stride 4B-aligned
                    # so their DVE ops pack 2 fp16/cycle.  d3 reuses the
                    # odd plane at +1 (misaligned, slower op) - cheaper
                    # than a 4th plane's DMA volume or GpSimd's SBUF-port
                    # contention with DVE.
                    xk = xp[par]
                    d = [xk[:, kt, r0:r0 + nr, 0, 0:NT],
                         xk[:, kt, r0:r0 + nr, 1, 0:NT],
                         xk[:, kt, r0:r0 + nr, 2, 0:NT],
                         xk[:, kt, r0:r0 + nr, 1, 1:NT + 1]]
                    o = [tw[:, kt, j, r0:r0 + nr, :] for j in range(NJ)]
                    nc.vector.tensor_sub(o[0], d[0], d[2])   # T0 = d0 - d2
                    nc.vector.tensor_add(o[1], d[1], d[2])   # T1 = d1 + d2
                    nc.vector.tensor_sub(o[2], d[2], d[1])   # T2 = d2 - d1
                    nc.vector.tensor_sub(o[3], d[1], d[3])   # T3 = d1 - d3

            # Head: image-0 bands interleaved with per-(mt,j) weight DMAs
            # in exactly the order the PE consumes them, racing PE warmup.
            wap = w_d.ap()

            def w_dma(mt, j):
                for kt in range(KT):
                    ring(kt).dma_start(w_sb[:, kt, mt, :, j],
                                       wap[:, kt, mt, :, j])

            band_dma(0, 0, *BANDS4[0])
            for j in range(NJ):
                w_dma(0, j)
            w_dma(1, 0)
            w_dma(1, 1)
            band_dma(0, 0, *BANDS4[1])
            w_dma(1, 2)
            w_dma(1, 3)
            band_dma(0, 0, *BANDS4[2])
            band_dma(0, 0, *BANDS4[3])
            nc.scalar.dma_start(b_sb[:], b_d.ap())
            transform_band(0, *BANDS4[0])

            for n in range(N_PER_CORE):
                par = n % 2
                npar = (n + 1) % 2
                nxt = n + 1 < N_PER_CORE
                chunks = [(c, c * RB, RB) for c in range(NCH)]
                if n == N_PER_CORE - 1:
                    # split the final chunk so the closing inverse + out-DMA
                    # drain after the last matmul is half as long
                    chunks = chunks[:-1] + [(NCH - 1, 42, 7), (NCH - 1, 49, 7)]
                for c, y0, rb in chunks:
                    # just-in-time transforms / prefetch DMAs, interleaved
                    # into the DVE stream between inverse-transform ops:
                    # rows 0..29 of image n are transformed during image
                    # n-1 (c2/c3 slots), rows 30..57 at (n, c0).  Keeping
                    # transforms early also releases the xp WAR quickly so
                    # the next image's DMA enqueues never head-of-line
                    # block the ACT queue.
                    if c == 0:
                        transform_band(par, *BANDS4[1])
                        if nxt:
                            for bnd in DMA_BANDS:
                                band_dma(n + 1, npar, *bnd)
                    elif c == 1:
                        transform_band(par, *BANDS4[2])
                    elif c == 2:
                        transform_band(par, *BANDS4[3])
                    elif c == 3 and nxt:
                        transform_band(npar, *BANDS4[0])
                    for mt in range(MT):
                        ps = [psum_pool.tile([128, RB, NT], fp32,
                                             name="ps", tag="ps")
                              for _ in range(NJ)]
                        for j in range(NJ):
                            first = True
                            for kt in range(KT):
                                for dy in range(K1):
                                    last = (kt == KT - 1 and dy == K1 - 1)
                                    nc.tensor.matmul(
                                        ps[j][:, 0:rb, :],
                                        w_sb[:, kt, mt, dy, j, :],
                                        tws[par][:, kt, j,
                                                 y0 + dy:y0 + dy + rb, :],
                                        start=first, stop=last,
                                    )
                                    first = False
                        # inverse transform + bias.  ACT stages the PSUM
                        # banks to fp16 SBUF (bias folded into M1, which
                        # appears with + sign in both output phases); DVE
                        # then runs 4 all-fp16 unit-stride ops at 2x:
                        #   y_even = M0 + (M1+b) + M2
                        #   y_odd  = (M1+b) - M2 - M3
                        # ot is phase-split [rows, 2, 28]; host interleaves.
                        ot = out_pool.tile([128, RB, 2, NT], fp16,
                                           name="ot", tag="ot")
                        cc = [tmp_pool.tile([128, RB, NT], fp16,
                                            name=f"c{k}", tag=f"c{k}")
                              for k in range(NJ)]
                        nc.scalar.copy(cc[0][:, 0:rb], ps[0][:, 0:rb])
                        nc.scalar.add(cc[1][:, 0:rb], ps[1][:, 0:rb],
                                      b_sb[:, mt:mt + 1])
                        nc.scalar.copy(cc[2][:, 0:rb], ps[2][:, 0:rb])
                        nc.scalar.copy(cc[3][:, 0:rb], ps[3][:, 0:rb])
                        e1 = tmp_pool.tile([128, RB, NT], fp16,
                                           name="e1", tag="e1")
                        u1 = tmp_pool.tile([128, RB, NT], fp16,
                                           name="u1", tag="u1")
                        nc.vector.tensor_add(e1[:, 0:rb], cc[0][:, 0:rb],
                                             cc[1][:, 0:rb])
                        nc.vector.tensor_add(ot[:, 0:rb, 0, :], e1[:, 0:rb],
                                             cc[2][:, 0:rb])
                        nc.vector.tensor_sub(u1[:, 0:rb], cc[1][:, 0:rb],
                                             cc[2][:, 0:rb])
                        nc.vector.tensor_sub(ot[:, 0:rb, 1, :], u1[:, 0:rb],
                                             cc[3][:, 0:rb])
                        # out DMA: mt0 on the sync ring, mt1 on the gpsimd
                        # ring (keeps the ACT queue free of DMA waits; only
                        # SP/Activation/gpsimd can initiate DMAs)
                        r0 = y0 - c * RB
                        (nc.sync if mt == 0 else nc.gpsimd).dma_start(
                            oap[n, mt, c, :, r0:r0 + rb], ot[:, 0:rb])
    nc.compile()
    return nc


def get_nc():
    if "nc" not in _NC_CACHE:
        _NC_CACHE["nc"] = _build_nc()
    return _NC_CACHE["nc"]


def kernel(x, weight, alphas, betas, bias):
    from concourse.bass_utils import run_bass_kernel_spmd

    xpad, w_arr, b_arr = _host_prep(x, weight, alphas, betas, bias)
    nc = get_nc()
    in_maps = [
        {"x": xpad[i * N_PER_CORE:(i + 1) * N_PER_CORE], "w": w_arr,
         "b": b_arr}
        for i in range(N_CORES)
    ]
    res = run_bass_kernel_spmd(nc, in_maps, core_ids=list(range(N_CORES)))
    LAST_RESULT["res"] = res
    out = np.concatenate([r["out"] for r in res.results], axis=0)
    # [32, MT, NCH, 128, RB, 2, NT] -> [32, 256, 56, 56]
    # (x = 2*t + phase: transpose phase behind the tile index)
    return np.ascontiguousarray(
        out.transpose(0, 1, 3, 2, 4, 6, 5).reshape(BATCH, C, H, W)
    ).astype(np.float32)


# revision 26
# speedup vs baseline: 1.0246x; 1.0136x over previous
"""Trainium2 Bass kernel: 3x3 conv (NCHW 32x256x56x56, 256->256ch, pad 1) with
host-expanded synthesized weight, data-parallel over 8 NeuronCores.

1D Winograd F(2,3) along W cuts PE matmul work to 2/3 of direct implicit
GEMM: per 2 output columns, 4 winograd positions x (3 dy x 2 ch-tiles)
= 24 col-cycles vs 36 direct.  PE floor ~125us vs ~188us direct fp16
(fp8 DoubleRow would be 2x faster still but e4m3 on even one conv
operand measures 2.7e-2 max-rel error vs the 2e-2 gate - dead end).

Pipeline per image (4 per core): input arrives as column-parity planes
(even/odd/even-shifted, host-prepared) so DVE's 4 transform ops per
(kt, row-band) T0=d0-d2, T1=d1+d2, T2=d2-d1, T3=d1-d3 are unit-stride
fp16 and mostly run in the 2x packed mode; PE accumulates
M_j[co, r, t] = sum_{kt,dy} Gw[:,dy,j]^T T_j[:, r+dy, :] into 4 PSUM
banks per (14-row chunk, mt), 2 chunk-mt in flight (8 banks); ACT
stages the banks to fp16 SBUF (bias folded into M1, which carries a +
sign in both output phases); DVE finishes the inverse transform
y_even = M0+(M1+b)+M2, y_odd = (M1+b)-M2-M3 with 4 all-fp16 2x ops
into a phase-split fp16 out tile; host interleaves phases + casts fp32.

Measured ~163-170us per core (baseline direct implicit GEMM: ~228us);
PE matmul stream ~134us (84% occupancy), DVE ~90us, ACT ~75us, plus
~7.5us fixed NEFF preamble and ~12us inverse/DMA-drain/teardown tail.
"""

import numpy as np

# Problem constants (hardcoded per contract; kernel.py must be self-contained)
OOC, OIC, K1, K2 = 64, 64, 3, 3
R0, R1 = 4, 4
N_CORES = 8
BATCH = 32
N_PER_CORE = BATCH // N_CORES  # 4
C = 256
H = W = 56
HP = WP = H + 2  # zero-padded spatial (padding applied on host)
KT = C // 128    # 2 input-channel tiles
MT = C // 128    # 2 output-channel tiles
NT = 28          # winograd tiles along W (4-wide windows, stride 2)
NJ = 4           # winograd positions
RB = 14          # output rows per chunk -> psum bank [128, 14, 28] = 392 fp32
NCH = H // RB    # 4 chunks
# transform/DMA row bands (transform is row-local so any partition works);
# image 0 uses 4 chunk-aligned bands for fast start, later images 3 bands
# spread across chunk slots so input DMA (~19us/image) hides behind compute
BANDS4 = [(0, 16), (16, 14), (30, 14), (44, 14)]
TB_A0, TB_A1, TB_B = (0, 16), (16, 14), (30, 28)
DMA_BANDS = [TB_A0, TB_A1, TB_B]

_NC_CACHE = {}
LAST_RESULT = {}  # test.py introspection: last BassKernelResults

# F(2,3): G rows -> Gw_j weights; B^T rows -> T_j transforms
_G = np.array([[1, 0, 0], [0.5, 0.5, 0.5], [0.5, -0.5, 0.5], [0, 0, 1]],
              dtype=np.float32)


def _expand_weight(weight, alphas, betas):
    """W[p0*64+i, p1*64+j, ky, kx] = w[i,j,ky,kx] * a[p0,p1] / (1+exp(w*b[p0,p1]))."""
    w = weight.astype(np.float32)[None, None]            # (1,1,64,64,3,3)
    a = alphas.astype(np.float32).reshape(R0, R1)[:, :, None, None, None, None]
    b = betas.astype(np.float32).reshape(R0, R1)[:, :, None, None, None, None]
    act = w * a / (1.0 + np.exp(w * b))                  # (4,4,64,64,3,3)
    return act.transpose(0, 2, 1, 3, 4, 5).reshape(R0 * OOC, R1 * OIC, K1, K2)


def _host_prep(x, weight, alphas, betas, bias):
    x = np.asarray(x, dtype=np.float32).astype(np.float16)
    xpad = np.pad(x, ((0, 0), (0, 0), (1, 1), (1, 1)))
    xpad = xpad.reshape(BATCH, KT, 128, HP, WP)
    # column-parity planes, 30 elems each (4B-aligned unit-stride taps so
    # every DVE transform op hits the 2x packed mode):
    #   plane0 = even cols e0.., plane1 = odd cols o0..,
    #   plane2 = even shifted left (e1..), plane3 = odd shifted (o1..)
    xp3 = np.zeros((BATCH, KT, 128, HP, 3, 30), np.float16)
    xp3[..., 0, :29] = xpad[..., 0::2]
    xp3[..., 1, :29] = xpad[..., 1::2]
    xp3[..., 2, :28] = xpad[..., 2::2]
    xpad = xp3
    Wfull = _expand_weight(np.asarray(weight), np.asarray(alphas),
                           np.asarray(betas))            # (256,256,3,3) co,ci
    # winograd weights: Gw[ci, co, dy, j] = sum_k G[j,k] * W[co, ci, dy, k]
    Wt = Wfull.transpose(1, 0, 2, 3)                     # (ci, co, dy, kx)
    Gw = np.einsum("jk,iodk->iodj", _G, Wt)              # (ci, co, dy, 4)
    # lhsT layout: [ci_local(128), kt, mt, dy, j, co_local(128)]
    w_arr = np.ascontiguousarray(
        Gw.reshape(KT, 128, MT, 128, K1, NJ).transpose(1, 0, 2, 4, 5, 3)
    ).astype(np.float16)
    b_arr = np.ascontiguousarray(
        np.asarray(bias, dtype=np.float32).reshape(MT, 128).T)
    return xpad, w_arr, b_arr


def _build_nc():
    import concourse.mybir as mybir
    import concourse.tile as tile
    from concourse import bacc

    fp32 = mybir.dt.float32
    fp16 = mybir.dt.float16

    nc = bacc.Bacc("TRN2", target_bir_lowering=False, debug=False,
                   num_devices=N_CORES)

    x_d = nc.dram_tensor("x", [N_PER_CORE, KT, 128, HP, 3, 30], fp16,
                         kind="ExternalInput")
    w_d = nc.dram_tensor("w", [128, KT, MT, K1, NJ, 128], fp16,
                         kind="ExternalInput")
    b_d = nc.dram_tensor("b", [128, MT], fp32, kind="ExternalInput")
    o_d = nc.dram_tensor("out", [N_PER_CORE, MT, NCH, 128, RB, 2, NT], fp16,
                         kind="ExternalOutput")

    # Two HWDGE rings: sync carries kt=0 traffic, scalar carries kt=1.
    def ring(i):
        return nc.sync if i == 0 else nc.scalar

    with tile.TileContext(nc) as tc:
        with (
            tc.tile_pool(name="const", bufs=1) as const_pool,
            tc.tile_pool(name="xpad", bufs=1) as xp_pool,
            tc.tile_pool(name="tw", bufs=1) as tw_pool,
            tc.tile_pool(name="tmp", bufs=4) as tmp_pool,
            tc.tile_pool(name="ot", bufs=8) as out_pool,
            tc.tile_pool(name="ps", bufs=8, space="PSUM") as psum_pool,
        ):
            w_sb = const_pool.tile([128, KT, MT, K1, NJ, 128], fp16,
                                   name="w_sb", tag="w_sb")
            b_sb = const_pool.tile([128, MT], fp32, name="b_sb", tag="b_sb")

            # PE warmup: junk matmuls on scratch SBUF while the head DMAs
            # land flip the HAM clock gate to 8/8 before the real stream.
            warm_in = const_pool.tile([128, 128], fp16, name="warm_in",
                                      tag="warm_in")
            warm_ps = psum_pool.tile([128, RB, NT], fp32, name="warm_ps",
                                     tag="ps")
            nc.vector.memset(warm_in[:], 0.0)
            for _ in range(160):
                nc.tensor.matmul(warm_ps[:, 0, :], warm_in[:],
                                 warm_in[:, 0:NT])

            # Double-buffered padded input images and winograd-domain input.
            # T layout: [128, kt, j, 58 rows, 28 tiles] fp16.
            xp = [xp_pool.tile([128, KT, HP, 3, 30], fp16, name=f"xp{par}",
                               tag=f"xp{par}") for par in range(2)]
            tws = [tw_pool.tile([128, KT, NJ, HP, NT], fp16, name=f"tw{par}",
                                tag=f"tw{par}") for par in range(2)]

            xap = x_d.ap()
            oap = o_d.ap()

            def band_dma(n, par, r0, nr):
                for kt in range(KT):
                    ring(kt).dma_start(
                        xp[par][:, kt, r0:r0 + nr, :, :],
                        xap[n, kt, :, r0:r0 + nr, :, :])

            def transform_band(par, r0, nr):
                tw = tws[par]
                for kt in range(KT):
                    # parity planes: d0 = even cols, d1 = odd, d2 = even
                    # shifted left; those taps are unit-stride 4B-aligned
                    # so their DVE ops pack 2 fp16/cycle.  d3 reuses the
                    # odd plane at +1 (misaligned, slower op) - cheaper
                    # than a 4th plane's DMA volume or GpSimd's SBUF-port
                    # contention with DVE.
                    xk = xp[par]
                    d = [xk[:, kt, r0:r0 + nr, 0, 0:NT],
                         xk[:, kt, r0:r0 + nr, 1, 0:NT],
                         xk[:, kt, r0:r0 + nr, 2, 0:NT],
                         xk[:, kt, r0:r0 + nr, 1, 1:NT + 1]]
                    o = [tw[:, kt, j, r0:r0 + nr, :] for j in range(NJ)]
                    nc.vector.tensor_sub(o[0], d[0], d[2])   # T0 = d0 - d2
                    nc.vector.tensor_add(o[1], d[1], d[2])   # T1 = d1 + d2
                    nc.vector.tensor_sub(o[2], d[2], d[1])   # T2 = d2 - d1
                    nc.vector.tensor_sub(o[3], d[1], d[3])   # T3 = d1 - d3

            # Head: image-0 bands interleaved with per-(mt,j) weight DMAs
            # in exactly the order the PE consumes them, racing PE warmup.
            wap = w_d.ap()

            def w_dma(mt, j):
                for kt in range(KT):
                    ring(kt).dma_start(w_sb[:, kt, mt, :, j],
                                       wap[:, kt, mt, :, j])

            band_dma(0, 0, *BANDS4[0])
            # image-0 bands 2+3 ride the gpsimd ring, idle during the head
            # (its first out-DMA enqueue comes ~12us in) - sync/scalar then
            # deliver band0/1 + all weights ~6us sooner
            for r0, nr in (BANDS4[2], BANDS4[3]):
                for kt in range(KT):
                    nc.gpsimd.dma_start(
                        xp[0][:, kt, r0:r0 + nr, :, :],
                        xap[0, kt, :, r0:r0 + nr, :, :])
            for j in range(NJ):
                w_dma(0, j)
            w_dma(1, 0)
            w_dma(1, 1)
            band_dma(0, 0, *BANDS4[1])
            w_dma(1, 2)
            w_dma(1, 3)
            nc.scalar.dma_start(b_sb[:], b_d.ap())
            transform_band(0, *BANDS4[0])

            for n in range(N_PER_CORE):
                par = n % 2
                npar = (n + 1) % 2
                nxt = n + 1 < N_PER_CORE
                chunks = [(c, c * RB, RB) for c in range(NCH)]
                if n == N_PER_CORE - 1:
                    # split the final chunk so the closing inverse + out-DMA
                    # drain after the last matmul is half as long
                    chunks = chunks[:-1] + [(NCH - 1, 42, 7), (NCH - 1, 49, 7)]
                for c, y0, rb in chunks:
                    # just-in-time transforms / prefetch DMAs, interleaved
                    # into the DVE stream between inverse-transform ops:
                    # rows 0..29 of image n are transformed during image
                    # n-1 (c2/c3 slots), rows 30..57 at (n, c0).  Keeping
                    # transforms early also releases the xp WAR quickly so
                    # the next image's DMA enqueues never head-of-line
                    # block the ACT queue.
                    if c == 0:
                        transform_band(par, *BANDS4[1])
                        if nxt:
                            for bnd in DMA_BANDS:
                                band_dma(n + 1, npar, *bnd)
                    elif c == 1:
                        transform_band(par, *BANDS4[2])
                    elif c == 2:
                        transform_band(par, *BANDS4[3])
                    elif c == 3 and nxt:
                        transform_band(npar, *BANDS4[0])
                    for mt in range(MT):
                        ps = [psum_pool.tile([128, RB, NT], fp32,
                                             name="ps", tag="ps")
                              for _ in range(NJ)]
                        for j in range(NJ):
                            first = True
                            for kt in range(KT):
                                for dy in range(K1):
                                    last = (kt == KT - 1 and dy == K1 - 1)
                                    nc.tensor.matmul(
                                        ps[j][:, 0:rb, :],
                                        w_sb[:, kt, mt, dy, j, :],
                                        tws[par][:, kt, j,
                                                 y0 + dy:y0 + dy + rb, :],
                                        start=first, stop=last,
                                    )
                                    first = False
                        # inverse transform + bias.  ACT stages the PSUM
                        # banks to fp16 SBUF (bias folded into M1, which
                        # appears with + sign in both output phases); DVE
                        # then runs 4 all-fp16 unit-stride ops at 2x:
                        #   y_even = M0 + (M1+b) + M2
                        #   y_odd  = (M1+b) - M2 - M3
                        # ot is phase-split [rows, 2, 28]; host interleaves.
                        ot = out_pool.tile([128, RB, 2, NT], fp16,
                                           name="ot", tag="ot")
                        cc = [tmp_pool.tile([128, RB, NT], fp16,
                                            name=f"c{k}", tag=f"c{k}")
                              for k in range(NJ)]
                        nc.scalar.copy(cc[0][:, 0:rb], ps[0][:, 0:rb])
                        nc.scalar.add(cc[1][:, 0:rb], ps[1][:, 0:rb],
                                      b_sb[:, mt:mt + 1])
                        nc.scalar.copy(cc[2][:, 0:rb], ps[2][:, 0:rb])
                        nc.scalar.copy(cc[3][:, 0:rb], ps[3][:, 0:rb])
                        e1 = tmp_pool.tile([128, RB, NT], fp16,
                                           name="e1", tag="e1")
                        u1 = tmp_pool.tile([128, RB, NT], fp16,
                                           name="u1", tag="u1")
                        nc.vector.tensor_add(e1[:, 0:rb], cc[0][:, 0:rb],
                                             cc[1][:, 0:rb])
                        nc.vector.tensor_add(ot[:, 0:rb, 0, :], e1[:, 0:rb],
                                             cc[2][:, 0:rb])
                        nc.vector.tensor_sub(u1[:, 0:rb], cc[1][:, 0:rb],
                                             cc[2][:, 0:rb])
                        nc.vector.tensor_sub(ot[:, 0:rb, 1, :], u1[:, 0:rb],
                                             cc[3][:, 0:rb])
                        # out DMA: mt0 on the sync ring, mt1 on the gpsimd
                        # ring (keeps the ACT queue free of DMA waits; only
                        # SP/Activation/gpsimd can initiate DMAs)
                        r0 = y0 - c * RB
                        (nc.sync if mt == 0 else nc.gpsimd).dma_start(
                            oap[n, mt, c, :, r0:r0 + rb], ot[:, 0:rb])
    nc.compile()
    return nc


def get_nc():
    if "nc" not in _NC_CACHE:
        _NC_CACHE["nc"] = _build_nc()
    return _NC_CACHE["nc"]


def kernel(x, weight, alphas, betas, bias):
    from concourse.bass_utils import run_bass_kernel_spmd

    xpad, w_arr, b_arr = _host_prep(x, weight, alphas, betas, bias)
    nc = get_nc()
    in_maps = [
        {"x": xpad[i * N_PER_CORE:(i + 1) * N_PER_CORE], "w": w_arr,
         "b": b_arr}
        for i in range(N_CORES)
    ]
    res = run_bass_kernel_spmd(nc, in_maps, core_ids=list(range(N_CORES)))
    LAST_RESULT["res"] = res
    out = np.concatenate([r["out"] for r in res.results], axis=0)
    # [32, MT, NCH, 128, RB, 2, NT] -> [32, 256, 56, 56]
    # (x = 2*t + phase: transpose phase behind the tile index)
    return np.ascontiguousarray(
        out.transpose(0, 1, 3, 2, 4, 6, 5).reshape(BATCH, C, H, W)
    ).astype(np.float32)


# revision 27
# speedup vs baseline: 1.0247x; 1.0001x over previous
"""Trainium2 Bass kernel: 3x3 conv (NCHW 32x256x56x56, 256->256ch, pad 1) with
host-expanded synthesized weight, data-parallel over 8 NeuronCores.

1D Winograd F(2,3) along W cuts PE matmul work to 2/3 of direct implicit
GEMM: per 2 output columns, 4 winograd positions x (3 dy x 2 ch-tiles)
= 24 col-cycles vs 36 direct.  PE floor ~125us vs ~188us direct fp16
(fp8 DoubleRow would be 2x faster still but e4m3 on even one conv
operand measures 2.7e-2 max-rel error vs the 2e-2 gate - dead end).

Pipeline per image (4 per core): input arrives as column-parity planes
(even/odd/even-shifted, host-prepared) so DVE's 4 transform ops per
(kt, row-band) T0=d0-d2, T1=d1+d2, T2=d2-d1, T3=d1-d3 are unit-stride
fp16 and mostly run in the 2x packed mode; PE accumulates
M_j[co, r, t] = sum_{kt,dy} Gw[:,dy,j]^T T_j[:, r+dy, :] into 4 PSUM
banks per (14-row chunk, mt), 2 chunk-mt in flight (8 banks); ACT
stages the banks to fp16 SBUF (bias folded into M1, which carries a +
sign in both output phases); DVE finishes the inverse transform
y_even = M0+(M1+b)+M2, y_odd = (M1+b)-M2-M3 with 4 all-fp16 2x ops
into a phase-split fp16 out tile; host interleaves phases + casts fp32.

Measured ~161-167us per core (baseline direct implicit GEMM: ~228us);
PE matmul stream ~134us (84% occupancy), DVE ~90us, ACT ~75us, plus
~7.5us fixed NEFF preamble and ~12us inverse/DMA-drain/teardown tail.
Head decongestion matters: image-0 bands 2-3 ride the gpsimd DMA ring
(idle during the head), extended PE warmup covers the clock-gated wait,
and the last image's final chunk is split 7+7 to shorten the closing
inverse + DMA drain.
"""

import numpy as np

# Problem constants (hardcoded per contract; kernel.py must be self-contained)
OOC, OIC, K1, K2 = 64, 64, 3, 3
R0, R1 = 4, 4
N_CORES = 8
BATCH = 32
N_PER_CORE = BATCH // N_CORES  # 4
C = 256
H = W = 56
HP = WP = H + 2  # zero-padded spatial (padding applied on host)
KT = C // 128    # 2 input-channel tiles
MT = C // 128    # 2 output-channel tiles
NT = 28          # winograd tiles along W (4-wide windows, stride 2)
NJ = 4           # winograd positions
RB = 14          # output rows per chunk -> psum bank [128, 14, 28] = 392 fp32
NCH = H // RB    # 4 chunks
# transform/DMA row bands (transform is row-local so any partition works);
# image 0 uses 4 chunk-aligned bands for fast start, later images 3 bands
# spread across chunk slots so input DMA (~19us/image) hides behind compute
BANDS4 = [(0, 16), (16, 14), (30, 14), (44, 14)]
TB_A0, TB_A1, TB_B = (0, 16), (16, 14), (30, 28)
DMA_BANDS = [TB_A0, TB_A1, TB_B]

_NC_CACHE = {}
LAST_RESULT = {}  # test.py introspection: last BassKernelResults

# F(2,3): G rows -> Gw_j weights; B^T rows -> T_j transforms
_G = np.array([[1, 0, 0], [0.5, 0.5, 0.5], [0.5, -0.5, 0.5], [0, 0, 1]],
              dtype=np.float32)


def _expand_weight(weight, alphas, betas):
    """W[p0*64+i, p1*64+j, ky, kx] = w[i,j,ky,kx] * a[p0,p1] / (1+exp(w*b[p0,p1]))."""
    w = weight.astype(np.float32)[None, None]            # (1,1,64,64,3,3)
    a = alphas.astype(np.float32).reshape(R0, R1)[:, :, None, None, None, None]
    b = betas.astype(np.float32).reshape(R0, R1)[:, :, None, None, None, None]
    act = w * a / (1.0 + np.exp(w * b))                  # (4,4,64,64,3,3)
    return act.transpose(0, 2, 1, 3, 4, 5).reshape(R0 * OOC, R1 * OIC, K1, K2)


def _host_prep(x, weight, alphas, betas, bias):
    x = np.asarray(x, dtype=np.float32).astype(np.float16)
    xpad = np.pad(x, ((0, 0), (0, 0), (1, 1), (1, 1)))
    xpad = xpad.reshape(BATCH, KT, 128, HP, WP)
    # column-parity planes, 30 elems each (4B-aligned unit-stride taps so
    # every DVE transform op hits the 2x packed mode):
    #   plane0 = even cols e0.., plane1 = odd cols o0..,
    #   plane2 = even shifted left (e1..), plane3 = odd shifted (o1..)
    xp3 = np.zeros((BATCH, KT, 128, HP, 3, 30), np.float16)
    xp3[..., 0, :29] = xpad[..., 0::2]
    xp3[..., 1, :29] = xpad[..., 1::2]
    xp3[..., 2, :28] = xpad[..., 2::2]
    xpad = xp3
    Wfull = _expand_weight(np.asarray(weight), np.asarray(alphas),
                           np.asarray(betas))            # (256,256,3,3) co,ci
    # winograd weights: Gw[ci, co, dy, j] = sum_k G[j,k] * W[co, ci, dy, k]
    Wt = Wfull.transpose(1, 0, 2, 3)                     # (ci, co, dy, kx)
    Gw = np.einsum("jk,iodk->iodj", _G, Wt)              # (ci, co, dy, 4)
    # lhsT layout: [ci_local(128), kt, mt, dy, j, co_local(128)]
    w_arr = np.ascontiguousarray(
        Gw.reshape(KT, 128, MT, 128, K1, NJ).transpose(1, 0, 2, 4, 5, 3)
    ).astype(np.float16)
    b_arr = np.ascontiguousarray(
        np.asarray(bias, dtype=np.float32).reshape(MT, 128).T)
    return xpad, w_arr, b_arr


def _build_nc():
    import concourse.mybir as mybir
    import concourse.tile as tile
    from concourse import bacc

    fp32 = mybir.dt.float32
    fp16 = mybir.dt.float16

    nc = bacc.Bacc("TRN2", target_bir_lowering=False, debug=False,
                   num_devices=N_CORES)

    x_d = nc.dram_tensor("x", [N_PER_CORE, KT, 128, HP, 3, 30], fp16,
                         kind="ExternalInput")
    w_d = nc.dram_tensor("w", [128, KT, MT, K1, NJ, 128], fp16,
                         kind="ExternalInput")
    b_d = nc.dram_tensor("b", [128, MT], fp32, kind="ExternalInput")
    o_d = nc.dram_tensor("out", [N_PER_CORE, MT, NCH, 128, RB, 2, NT], fp16,
                         kind="ExternalOutput")

    # Two HWDGE rings: sync carries kt=0 traffic, scalar carries kt=1.
    def ring(i):
        return nc.sync if i == 0 else nc.scalar

    with tile.TileContext(nc) as tc:
        with (
            tc.tile_pool(name="const", bufs=1) as const_pool,
            tc.tile_pool(name="xpad", bufs=1) as xp_pool,
            tc.tile_pool(name="tw", bufs=1) as tw_pool,
            tc.tile_pool(name="tmp", bufs=4) as tmp_pool,
            tc.tile_pool(name="ot", bufs=8) as out_pool,
            tc.tile_pool(name="ps", bufs=8, space="PSUM") as psum_pool,
        ):
            w_sb = const_pool.tile([128, KT, MT, K1, NJ, 128], fp16,
                                   name="w_sb", tag="w_sb")
            b_sb = const_pool.tile([128, MT], fp32, name="b_sb", tag="b_sb")

            # PE warmup: junk matmuls on scratch SBUF while the head DMAs
            # land flip the HAM clock gate to 8/8 before the real stream.
            warm_in = const_pool.tile([128, 128], fp16, name="warm_in",
                                      tag="warm_in")
            warm_ps = psum_pool.tile([128, RB, NT], fp32, name="warm_ps",
                                     tag="ps")
            nc.vector.memset(warm_in[:], 0.0)
            for _ in range(160):
                nc.tensor.matmul(warm_ps[:, 0, :], warm_in[:],
                                 warm_in[:, 0:NT])

            # Double-buffered padded input images and winograd-domain input.
            # T layout: [128, kt, j, 58 rows, 28 tiles] fp16.
            xp = [xp_pool.tile([128, KT, HP, 3, 30], fp16, name=f"xp{par}",
                               tag=f"xp{par}") for par in range(2)]
            tws = [tw_pool.tile([128, KT, NJ, HP, NT], fp16, name=f"tw{par}",
                                tag=f"tw{par}") for par in range(2)]

            xap = x_d.ap()
            oap = o_d.ap()

            def band_dma(n, par, r0, nr):
                for kt in range(KT):
                    ring(kt).dma_start(
                        xp[par][:, kt, r0:r0 + nr, :, :],
                        xap[n, kt, :, r0:r0 + nr, :, :])

            def transform_band(par, r0, nr):
                tw = tws[par]
                for kt in range(KT):
                    # parity planes: d0 = even cols, d1 = odd, d2 = even
                    # shifted left; those taps are unit-stride 4B-aligned
                    # so their DVE ops pack 2 fp16/cycle.  d3 reuses the
                    # odd plane at +1 (misaligned, slower op) - cheaper
                    # than a 4th plane's DMA volume or GpSimd's SBUF-port
                    # contention with DVE.
                    xk = xp[par]
                    d = [xk[:, kt, r0:r0 + nr, 0, 0:NT],
                         xk[:, kt, r0:r0 + nr, 1, 0:NT],
                         xk[:, kt, r0:r0 + nr, 2, 0:NT],
                         xk[:, kt, r0:r0 + nr, 1, 1:NT + 1]]
                    o = [tw[:, kt, j, r0:r0 + nr, :] for j in range(NJ)]
                    nc.vector.tensor_sub(o[0], d[0], d[2])   # T0 = d0 - d2
                    nc.vector.tensor_add(o[1], d[1], d[2])   # T1 = d1 + d2
                    nc.vector.tensor_sub(o[2], d[2], d[1])   # T2 = d2 - d1
                    nc.vector.tensor_sub(o[3], d[1], d[3])   # T3 = d1 - d3

            # Head: image-0 bands interleaved with per-(mt,j) weight DMAs
            # in exactly the order the PE consumes them, racing PE warmup.
            wap = w_d.ap()

            def w_dma(mt, j):
                for kt in range(KT):
                    ring(kt).dma_start(w_sb[:, kt, mt, :, j],
                                       wap[:, kt, mt, :, j])

            band_dma(0, 0, *BANDS4[0])
            # image-0 bands 2+3 ride the gpsimd ring, idle during the head
            # (its first out-DMA enqueue comes ~12us in) - sync/scalar then
            # deliver band0/1 + all weights ~6us sooner
            for r0, nr in (BANDS4[2], BANDS4[3]):
                for kt in range(KT):
                    nc.gpsimd.dma_start(
                        xp[0][:, kt, r0:r0 + nr, :, :],
                        xap[0, kt, :, r0:r0 + nr, :, :])
            for j in range(NJ):
                w_dma(0, j)
            w_dma(1, 0)
            w_dma(1, 1)
            band_dma(0, 0, *BANDS4[1])
            w_dma(1, 2)
            w_dma(1, 3)
            nc.scalar.dma_start(b_sb[:], b_d.ap())
            transform_band(0, *BANDS4[0])

            for n in range(N_PER_CORE):
                par = n % 2
                npar = (n + 1) % 2
                nxt = n + 1 < N_PER_CORE
                chunks = [(c, c * RB, RB) for c in range(NCH)]
                if n == N_PER_CORE - 1:
                    # split the final chunk so the closing inverse + out-DMA
                    # drain after the last matmul is half as long
                    chunks = chunks[:-1] + [(NCH - 1, 42, 7), (NCH - 1, 49, 7)]
                for c, y0, rb in chunks:
                    # just-in-time transforms / prefetch DMAs, interleaved
                    # into the DVE stream between inverse-transform ops:
                    # rows 0..29 of image n are transformed during image
                    # n-1 (c2/c3 slots), rows 30..57 at (n, c0).  Keeping
                    # transforms early also releases the xp WAR quickly so
                    # the next image's DMA enqueues never head-of-line
                    # block the ACT queue.
                    if c == 0:
                        transform_band(par, *BANDS4[1])
                        if nxt:
                            for bnd in DMA_BANDS:
                                band_dma(n + 1, npar, *bnd)
                    elif c == 1:
                        transform_band(par, *BANDS4[2])
                    elif c == 2:
                        transform_band(par, *BANDS4[3])
                    elif c == 3 and nxt:
                        transform_band(npar, *BANDS4[0])
                    for mt in range(MT):
                        ps = [psum_pool.tile([128, RB, NT], fp32,
                                             name="ps", tag="ps")
                              for _ in range(NJ)]
                        for j in range(NJ):
                            first = True
                            for kt in range(KT):
                                for dy in range(K1):
                                    last = (kt == KT - 1 and dy == K1 - 1)
                                    nc.tensor.matmul(
                                        ps[j][:, 0:rb, :],
                                        w_sb[:, kt, mt, dy, j, :],
                                        tws[par][:, kt, j,
                                                 y0 + dy:y0 + dy + rb, :],
                                        start=first, stop=last,
                                    )
                                    first = False
                        # inverse transform + bias.  ACT stages the PSUM
                        # banks to fp16 SBUF (bias folded into M1, which
                        # appears with + sign in both output phases); DVE
                        # then runs 4 all-fp16 unit-stride ops at 2x:
                        #   y_even = M0 + (M1+b) + M2
                        #   y_odd  = (M1+b) - M2 - M3
                        # ot is phase-split [rows, 2, 28]; host interleaves.
                        ot = out_pool.tile([128, RB, 2, NT], fp16,
                                           name="ot", tag="ot")
                        cc = [tmp_pool.tile([128, RB, NT], fp16,
                                            name=f"c{k}", tag=f"c{k}")
                              for k in range(NJ)]
                        nc.scalar.copy(cc[0][:, 0:rb], ps[0][:, 0:rb])
                        nc.scalar.add(cc[1][:, 0:rb], ps[1][:, 0:rb],
                                      b_sb[:, mt:mt + 1])
                        nc.scalar.copy(cc[2][:, 0:rb], ps[2][:, 0:rb])
                        nc.scalar.copy(cc[3][:, 0:rb], ps[3][:, 0:rb])
                        e1 = tmp_pool.tile([128, RB, NT], fp16,
                                           name="e1", tag="e1")
                        u1 = tmp_pool.tile([128, RB, NT], fp16,
                                           name="u1", tag="u1")
                        nc.vector.tensor_add(e1[:, 0:rb], cc[0][:, 0:rb],
                                             cc[1][:, 0:rb])
                        nc.vector.tensor_add(ot[:, 0:rb, 0, :], e1[:, 0:rb],
                                             cc[2][:, 0:rb])
                        nc.vector.tensor_sub(u1[:, 0:rb], cc[1][:, 0:rb],
                                             cc[2][:, 0:rb])
                        nc.vector.tensor_sub(ot[:, 0:rb, 1, :], u1[:, 0:rb],
                                             cc[3][:, 0:rb])
                        # out DMA: mt0 on the sync ring, mt1 on the gpsimd
                        # ring (keeps the ACT queue free of DMA waits; only
                        # SP/Activation/gpsimd can initiate DMAs)
                        r0 = y0 - c * RB
                        (nc.sync if mt == 0 else nc.gpsimd).dma_start(
                            oap[n, mt, c, :, r0:r0 + rb], ot[:, 0:rb])
    nc.compile()
    return nc


def get_nc():
    if "nc" not in _NC_CACHE:
        _NC_CACHE["nc"] = _build_nc()
    return _NC_CACHE["nc"]


def kernel(x, weight, alphas, betas, bias):
    from concourse.bass_utils import run_bass_kernel_spmd

    xpad, w_arr, b_arr = _host_prep(x, weight, alphas, betas, bias)
    nc = get_nc()
    in_maps = [
        {"x": xpad[i * N_PER_CORE:(i + 1) * N_PER_CORE], "w": w_arr,
         "b": b_arr}
        for i in range(N_CORES)
    ]
    res = run_bass_kernel_spmd(nc, in_maps, core_ids=list(range(N_CORES)))
    LAST_RESULT["res"] = res
    out = np.concatenate([r["out"] for r in res.results], axis=0)
    # [32, MT, NCH, 128, RB, 2, NT] -> [32, 256, 56, 56]
    # (x = 2*t + phase: transpose phase behind the tile index)
    return np.ascontiguousarray(
        out.transpose(0, 1, 3, 2, 4, 6, 5).reshape(BATCH, C, H, W)
    ).astype(np.float32)
